# revision 27
# baseline (speedup 1.0000x reference)
"""Trainium2 Bass kernel for ExcitationEmbedding + Ion RoPE.

Computes, for inputs
  excitations [256, 512, 2] int64 (pairs (a, b) with a, b in [0, 6)),
  n_electrons [256] f32, n_protons [256] f32,
  emb_weight  [26, 256] f32, lookup_table [6, 6] int64:

  idx   = lookup_table[a, b]                       # [B, N]
  emb   = emb_weight[idx]                          # [B, N, D]
  out   = per-batch block-diagonal rotation of emb (theta from n_electrons,
          phi from n_protons, 4-wide blocks: dims (0,1) by theta, (2,3) by phi)

Strategy (v4; pure data parallel over 8 cores, 32 batches each):
  - Host sends the token one-hot [36, BL*N] fp16 (pure index marshalling);
    the lut and emb tables are consumed on-device via a select-matmul that
    builds the 36-row fp16 tables e16 / esw16 (pair-swapped).
  - Per-batch rotated tables rot[j, d] = e16*C_b + esw16*S_b are built
    j-major in groups of 4 batches with 3 DVE ops per group; the C/S
    patterns reach all 36 partitions via one DRAM-bounce broadcast DMA
    covering all batches.
  - Gather: out_T[d_half, tok] = rot_slice.T @ onehot, fp16 matmuls with
    N=512 token streams, 2 per batch, weights and fmap both at partition 0.
  - PSUM pairs both halves in one [128, 1024] tile; evacuation (f32->fp16)
    alternates DVE/Act, weighted toward Act.
  - Output is fp16 in a [128, BL, 2, 512] d-major DRAM layout (8 KB
    contiguous per-partition packets, G=4 batches per sync-queue DMA); the
    host transposes back and converts to f32.
"""

import functools

import numpy as np

import concourse.bass as bass
import concourse.bacc as bacc
import concourse.mybir as mybir
from concourse import tile
from concourse.bass_utils import run_bass_kernel_spmd

B, N, D = 256, 512, 256
N_CORES = 8
BL = B // N_CORES   # 32 batches per core
G = 4               # batches per rot-group and per output DMA
ANGLE_SCALE = 0.05
HALF_PI = float(np.pi / 2)

F32 = mybir.dt.float32
F16 = mybir.dt.float16
AF = mybir.ActivationFunctionType
ALU = mybir.AluOpType


def build_bass() -> bass.Bass:
    nc = bacc.Bacc(
        "TRN2", target_bir_lowering=False, debug=False, num_devices=N_CORES
    )

    oh_in = nc.dram_tensor("oh", [36, BL * N], F16, kind="ExternalInput")
    ne = nc.dram_tensor("ne", [BL, 1], F32, kind="ExternalInput")
    npr = nc.dram_tensor("npr", [BL, 1], F32, kind="ExternalInput")
    emb = nc.dram_tensor("emb", [26, D], F32, kind="ExternalInput")
    lut = nc.dram_tensor("lut", [1, 36], F32, kind="ExternalInput")
    # out[p, b, h, n] = result[b, n, h*128 + p]
    out = nc.dram_tensor("out", [128, BL * 2 * N], F16, kind="ExternalOutput")

    iota_f32 = nc.inline_tensor(
        np.arange(36, dtype=np.float32).reshape(36, 1), "iota_f32")

    with tile.TileContext(nc) as tc:
        with (
            tc.tile_pool(name="const", bufs=1) as const,
            tc.tile_pool(name="bpool", bufs=3) as bpool,
            tc.tile_pool(name="opool", bufs=2) as opool,
            tc.tile_pool(name="dram", bufs=1, space="DRAM") as dram,
            tc.tile_pool(name="psum_s", bufs=1, space="PSUM") as psum_s,
            tc.tile_pool(name="psum", bufs=3, space="PSUM") as psum,
        ):
            # ---- loads (all on sync queue so Act starts computing at
            # once; sync is otherwise idle until the first output DMA) ----
            ne_bc = const.tile([36, BL], F32)
            nc.sync.dma_start(
                out=ne_bc[:],
                in_=ne[:].rearrange("q o -> (q o)").unsqueeze(0)
                .to_broadcast((36, BL)))
            npr_bc = const.tile([36, BL], F32)
            nc.sync.dma_start(
                out=npr_bc[:],
                in_=npr[:].rearrange("q o -> (q o)").unsqueeze(0)
                .to_broadcast((36, BL)))
            emb_f = const.tile([26, D], F32)
            nc.sync.dma_start(out=emb_f[:], in_=emb[:])
            lut_bc = const.tile([26, 36], F32)
            nc.sync.dma_start(out=lut_bc[:],
                              in_=lut[0:1, :].to_broadcast((26, 36)))
            oh_all = const.tile([36, BL * N], F16)
            quarter = BL * N // 4
            for c in range(4):
                sl = slice(c * quarter, (c + 1) * quarter)
                nc.sync.dma_start(out=oh_all[:, sl], in_=oh_in[:, sl])
            iota_s = const.tile([36, 1], F32)
            nc.gpsimd.iota(iota_s[:], pattern=[[0, 1]], base=0,
                           channel_multiplier=1,
                           allow_small_or_imprecise_dtypes=True)

            # ---- sin/cos pair tiles [36, BL, 2] fed straight from the
            # partition-broadcast ne/npr reads (no DRAM bounce) ----
            hp36 = const.tile([36, 1], F32)
            nc.vector.memset(hp36[:], HALF_PI)
            ctct = const.tile([36, BL, 2], F16)
            cpcp = const.tile([36, BL, 2], F16)
            stnst = const.tile([36, BL, 2], F16)
            spnsp = const.tile([36, BL, 2], F16)
            # cos(t) = sin(pi/2 - t) keeps the LUT argument within [-pi, pi]
            for t in range(2):
                nc.scalar.activation(ctct[:, :, t], ne_bc[:], AF.Sin,
                                     bias=hp36[:], scale=-ANGLE_SCALE)
                nc.scalar.activation(cpcp[:, :, t], npr_bc[:], AF.Sin,
                                     bias=hp36[:], scale=-ANGLE_SCALE)
            for t, sgn in enumerate([1.0, -1.0]):
                nc.scalar.activation(stnst[:, :, t], ne_bc[:], AF.Sin,
                                     bias=0.0, scale=sgn * ANGLE_SCALE)
                nc.scalar.activation(spnsp[:, :, t], npr_bc[:], AF.Sin,
                                     bias=0.0, scale=sgn * ANGLE_SCALE)

            # ---- 36-row fp16 tables via select-matmul ----
            emb16 = const.tile([26, D], F16)
            nc.vector.tensor_copy(emb16[:], emb_f[:])
            selT = const.tile([26, 36], F16)
            nc.vector.tensor_scalar(out=selT[:], in0=lut_bc[:],
                                    scalar1=iota_s[0:26, :], scalar2=None,
                                    op0=ALU.is_equal)
            eph_ps = psum_s.tile([36, D], F32)
            nc.tensor.matmul(eph_ps[:], selT[:], emb16[:], start=True,
                             stop=True)
            e16 = const.tile([36, D], F16)
            nc.vector.tensor_copy(e16[:], eph_ps[:])
            esw = const.tile([36, D], F16)
            e2 = e16[:].rearrange("j (k i) -> j k i", i=2)
            s2 = esw[:].rearrange("j (k i) -> j k i", i=2)
            nc.vector.tensor_copy(s2[:, :, 0], e2[:, :, 1])
            nc.vector.tensor_copy(s2[:, :, 1], e2[:, :, 0])

            # Act is faster per evac column but DVE carries the rot build:
            # 1 -> DVE, 0 -> Act (8 DVE / 24 Act out of 32)
            evac_dve = [0, 0, 1, 0]

            e4 = e16[:].rearrange("j (k i) -> j k i", i=4)
            w4 = esw[:].rearrange("j (k i) -> j k i", i=4)

            GR = 4   # batches per steady-state rot build

            def build_rot(b0, gr, tag):
                gs = slice(b0, b0 + gr)
                t1 = bpool.tile([36, gr, D], F16, tag="t1" + tag, bufs=4)
                t2 = bpool.tile([36, gr, D], F16, tag="t2" + tag, bufs=4)
                rot = bpool.tile([36, gr, D], F16, tag="rot" + tag, bufs=4)
                t14 = t1[:].rearrange("j q (k i) -> j q k i", i=4)
                t24 = t2[:].rearrange("j q (k i) -> j q k i", i=4)
                for lo, pair in ((0, ctct), (2, cpcp)):
                    nc.vector.tensor_mul(
                        t14[:, :, :, lo:lo + 2],
                        e4[:, :, lo:lo + 2].unsqueeze(1)
                        .to_broadcast((36, gr, 64, 2)),
                        pair[:, gs, :].unsqueeze(2)
                        .to_broadcast((36, gr, 64, 2)))
                for lo, pair in ((0, stnst), (2, spnsp)):
                    nc.vector.tensor_mul(
                        t24[:, :, :, lo:lo + 2],
                        w4[:, :, lo:lo + 2].unsqueeze(1)
                        .to_broadcast((36, gr, 64, 2)),
                        pair[:, gs, :].unsqueeze(2)
                        .to_broadcast((36, gr, 64, 2)))
                nc.vector.tensor_add(rot[:], t1[:], t2[:])
                return rot

            for b0 in range(0, BL, G):
                if b0 == 0:
                    # warm-up: two half groups so batch 0 starts sooner
                    rots = [(build_rot(0, 2, "w"), 2), (build_rot(2, 2, "w"), 2)]
                else:
                    rots = [(build_rot(b0, GR, ""), GR)]

                obuf = opool.tile([128, G * 2 * N], F16, tag="obuf", bufs=3)
                g = 0
                for rot, gr_n in rots:
                    for gr in range(gr_n):
                        b = b0 + g
                        # ---- gather: 2 fp16 matmuls, one 2-bank psum ----
                        ps = psum.tile([128, 2 * N], F32, tag="ps", bufs=3)
                        for h in range(2):
                            nc.tensor.matmul(ps[:, h * N:(h + 1) * N],
                                             rot[:, gr, h * 128:(h + 1) * 128],
                                             oh_all[:, b * N:(b + 1) * N],
                                             start=True, stop=True)
                        oslice = obuf[:, g * 2 * N:(g + 1) * 2 * N]
                        if evac_dve[b % 4]:
                            nc.vector.tensor_copy(oslice, ps[:])
                        else:
                            nc.scalar.activation(oslice, ps[:], AF.Copy)
                        if b0 + G >= BL:
                            # tail: fire each batch's write immediately
                            nc.sync.dma_start(
                                out=out[:, b * 2 * N:(b + 1) * 2 * N],
                                in_=oslice)
                        g += 1
                if b0 + G < BL:
                    nc.sync.dma_start(
                        out=out[:, b0 * 2 * N:(b0 + G) * 2 * N], in_=obuf[:])

    nc.compile()
    return nc


@functools.lru_cache(maxsize=1)
def _get_nc() -> bass.Bass:
    return build_bass()


def kernel_with_results(excitations, n_electrons, n_protons, emb_weight,
                        lookup_table, trace=False):
    exc = np.asarray(excitations)
    flat = (exc[..., 0] * 6 + exc[..., 1]).reshape(B, N)
    oh = (flat[:, None, :] == np.arange(36)[None, :, None]).astype(np.float16)
    ne = np.asarray(n_electrons, dtype=np.float32)
    npr = np.asarray(n_protons, dtype=np.float32)
    emb = np.ascontiguousarray(np.asarray(emb_weight, dtype=np.float32))
    lut_f = np.ascontiguousarray(
        np.asarray(lookup_table).astype(np.float32).reshape(1, 36))

    in_maps = []
    for c in range(N_CORES):
        sl = slice(c * BL, (c + 1) * BL)
        in_maps.append({
            "oh": np.ascontiguousarray(
                oh[sl].transpose(1, 0, 2).reshape(36, BL * N)),
            "ne": np.ascontiguousarray(ne[sl].reshape(BL, 1)),
            "npr": np.ascontiguousarray(npr[sl].reshape(BL, 1)),
            "emb": emb,
            "lut": lut_f,
        })

    nc = _get_nc()
    res = run_bass_kernel_spmd(nc, in_maps, list(range(N_CORES)), trace=trace)
    shards = []
    for c in range(N_CORES):
        arr = np.asarray(res.results[c]["out"]).reshape(128, BL, 2, N)
        shards.append(arr.transpose(1, 3, 2, 0).reshape(BL, N, D))
    out_arr = np.concatenate(shards, axis=0).astype(np.float32)
    return np.ascontiguousarray(out_arr), res


def kernel(excitations, n_electrons, n_protons, emb_weight, lookup_table):
    out_arr, _ = kernel_with_results(excitations, n_electrons, n_protons,
                                     emb_weight, lookup_table)
    return out_arr


# revision 28
# speedup vs baseline: 1.0298x; 1.0298x over previous
"""Trainium2 Bass kernel for ExcitationEmbedding + Ion RoPE.

Computes, for inputs
  excitations [256, 512, 2] int64 (pairs (a, b) with a, b in [0, 6)),
  n_electrons [256] f32, n_protons [256] f32,
  emb_weight  [26, 256] f32, lookup_table [6, 6] int64:

  idx   = lookup_table[a, b]                       # [B, N]
  emb   = emb_weight[idx]                          # [B, N, D]
  out   = per-batch block-diagonal rotation of emb (theta from n_electrons,
          phi from n_protons, 4-wide blocks: dims (0,1) by theta, (2,3) by phi)

Strategy (v4; pure data parallel over 8 cores, 32 batches each):
  - Host sends the token one-hot [36, BL*N] fp16 (pure index marshalling);
    the lut and emb tables are consumed on-device via a select-matmul that
    builds the 36-row fp16 tables e16 / esw16 (pair-swapped).
  - Per-batch rotated tables rot[j, d] = e16*C_b + esw16*S_b are built
    j-major in groups of 4 batches with 3 DVE ops per group; the C/S
    patterns reach all 36 partitions via one DRAM-bounce broadcast DMA
    covering all batches.
  - Gather: out_T[d_half, tok] = rot_slice.T @ onehot, fp16 matmuls with
    N=512 token streams, 2 per batch, weights and fmap both at partition 0.
  - PSUM pairs both halves in one [128, 1024] tile; evacuation (f32->fp16)
    alternates DVE/Act, weighted toward Act.
  - Output is fp16 in a [128, BL, 2, 512] d-major DRAM layout (8 KB
    contiguous per-partition packets, G=4 batches per sync-queue DMA); the
    host transposes back and converts to f32.
"""

import functools

import numpy as np

import concourse.bass as bass
import concourse.bacc as bacc
import concourse.mybir as mybir
from concourse import tile
from concourse.bass_utils import run_bass_kernel_spmd

B, N, D = 256, 512, 256
N_CORES = 8
BL = B // N_CORES   # 32 batches per core
G = 4               # batches per rot-group and per output DMA
ANGLE_SCALE = 0.05
HALF_PI = float(np.pi / 2)

F32 = mybir.dt.float32
F16 = mybir.dt.float16
AF = mybir.ActivationFunctionType
ALU = mybir.AluOpType


def build_bass() -> bass.Bass:
    nc = bacc.Bacc(
        "TRN2", target_bir_lowering=False, debug=False, num_devices=N_CORES
    )

    oh_in = nc.dram_tensor("oh", [36, BL * N], F16, kind="ExternalInput")
    ne = nc.dram_tensor("ne", [BL, 1], F32, kind="ExternalInput")
    npr = nc.dram_tensor("npr", [BL, 1], F32, kind="ExternalInput")
    emb = nc.dram_tensor("emb", [26, D], F32, kind="ExternalInput")
    lut = nc.dram_tensor("lut", [1, 36], F32, kind="ExternalInput")
    # out[p, b, h, n] = result[b, n, h*128 + p]
    out = nc.dram_tensor("out", [128, BL * 2 * N], F16, kind="ExternalOutput")

    iota_f32 = nc.inline_tensor(
        np.arange(36, dtype=np.float32).reshape(36, 1), "iota_f32")

    with tile.TileContext(nc) as tc:
        with (
            tc.tile_pool(name="const", bufs=1) as const,
            tc.tile_pool(name="bpool", bufs=3) as bpool,
            tc.tile_pool(name="opool", bufs=2) as opool,
            tc.tile_pool(name="dram", bufs=1, space="DRAM") as dram,
            tc.tile_pool(name="psum", bufs=4, space="PSUM") as psum,
        ):
            # ---- loads (all on sync queue so Act starts computing at
            # once; sync is otherwise idle until the first output DMA) ----
            ne_bc = const.tile([36, BL], F32)
            nc.sync.dma_start(
                out=ne_bc[:],
                in_=ne[:].rearrange("q o -> (q o)").unsqueeze(0)
                .to_broadcast((36, BL)))
            npr_bc = const.tile([36, BL], F32)
            nc.sync.dma_start(
                out=npr_bc[:],
                in_=npr[:].rearrange("q o -> (q o)").unsqueeze(0)
                .to_broadcast((36, BL)))
            emb_f = const.tile([26, D], F32)
            nc.sync.dma_start(out=emb_f[:], in_=emb[:])
            lut_bc = const.tile([26, 36], F32)
            nc.sync.dma_start(out=lut_bc[:],
                              in_=lut[0:1, :].to_broadcast((26, 36)))
            oh_all = const.tile([36, BL * N], F16)
            quarter = BL * N // 4
            for c in range(4):
                sl = slice(c * quarter, (c + 1) * quarter)
                nc.sync.dma_start(out=oh_all[:, sl], in_=oh_in[:, sl])
            iota_s = const.tile([36, 1], F32)
            nc.gpsimd.iota(iota_s[:], pattern=[[0, 1]], base=0,
                           channel_multiplier=1,
                           allow_small_or_imprecise_dtypes=True)

            # ---- sin/cos pair tiles [36, BL, 2] fed straight from the
            # partition-broadcast ne/npr reads (no DRAM bounce) ----
            hp36 = const.tile([36, 1], F32)
            nc.vector.memset(hp36[:], HALF_PI)
            pm2 = const.tile([36, 2], F32)
            nc.vector.memset(pm2[:, 0:1], ANGLE_SCALE)
            nc.vector.memset(pm2[:, 1:2], -ANGLE_SCALE)
            # dummy activation preloads the Sin table before ne/npr arrive
            scratch = const.tile([36, 1], F32)
            nc.scalar.activation(scratch[:], hp36[:], AF.Sin, bias=0.0,
                                 scale=1.0)
            nepm = const.tile([36, BL, 2], F32)
            nc.vector.tensor_mul(
                nepm[:], ne_bc[:].unsqueeze(2).to_broadcast((36, BL, 2)),
                pm2[:].unsqueeze(1).to_broadcast((36, BL, 2)))
            nppm = const.tile([36, BL, 2], F32)
            nc.vector.tensor_mul(
                nppm[:], npr_bc[:].unsqueeze(2).to_broadcast((36, BL, 2)),
                pm2[:].unsqueeze(1).to_broadcast((36, BL, 2)))
            ctct = const.tile([36, BL, 2], F16)
            cpcp = const.tile([36, BL, 2], F16)
            stnst = const.tile([36, BL, 2], F16)
            spnsp = const.tile([36, BL, 2], F16)
            # cos(t) = sin(pi/2 - t) keeps the LUT argument within [-pi, pi]
            nc.scalar.activation(
                ctct[:], ne_bc[:].unsqueeze(2).to_broadcast((36, BL, 2)),
                AF.Sin, bias=hp36[:], scale=-ANGLE_SCALE)
            nc.scalar.activation(
                cpcp[:], npr_bc[:].unsqueeze(2).to_broadcast((36, BL, 2)),
                AF.Sin, bias=hp36[:], scale=-ANGLE_SCALE)
            nc.scalar.activation(stnst[:], nepm[:], AF.Sin, bias=0.0,
                                 scale=1.0)
            nc.scalar.activation(spnsp[:], nppm[:], AF.Sin, bias=0.0,
                                 scale=1.0)

            # ---- 36-row fp16 tables via select-matmul ----
            emb16 = const.tile([26, D], F16)
            nc.vector.tensor_copy(emb16[:], emb_f[:])
            selT = const.tile([26, 36], F16)
            nc.vector.tensor_scalar(out=selT[:], in0=lut_bc[:],
                                    scalar1=iota_s[0:26, :], scalar2=None,
                                    op0=ALU.is_equal)
            eph_ps = psum.tile([128, 2 * N], F32, tag="ps", bufs=4)
            nc.tensor.matmul(eph_ps[0:36, 0:D], selT[:], emb16[:], start=True,
                             stop=True)
            e16 = const.tile([36, D], F16)
            nc.vector.tensor_copy(e16[:], eph_ps[0:36, 0:D])
            esw = const.tile([36, D], F16)
            e2 = e16[:].rearrange("j (k i) -> j k i", i=2)
            s2 = esw[:].rearrange("j (k i) -> j k i", i=2)
            nc.vector.tensor_copy(s2[:, :, 0], e2[:, :, 1])
            nc.vector.tensor_copy(s2[:, :, 1], e2[:, :, 0])

            # Act is faster per evac column but DVE carries the rot build:
            # 1 -> DVE, 0 -> Act (8 DVE / 24 Act out of 32)
            evac_dve = [0, 0, 1, 0]

            e4 = e16[:].rearrange("j (k i) -> j k i", i=4)
            w4 = esw[:].rearrange("j (k i) -> j k i", i=4)

            GR = 4   # batches per steady-state rot build

            def build_rot(b0, gr, tag):
                gs = slice(b0, b0 + gr)
                t1 = bpool.tile([36, gr, D], F16, tag="t1" + tag, bufs=4)
                t2 = bpool.tile([36, gr, D], F16, tag="t2" + tag, bufs=4)
                rot = bpool.tile([36, gr, D], F16, tag="rot" + tag, bufs=4)
                t14 = t1[:].rearrange("j q (k i) -> j q k i", i=4)
                t24 = t2[:].rearrange("j q (k i) -> j q k i", i=4)
                for lo, pair in ((0, ctct), (2, cpcp)):
                    nc.vector.tensor_mul(
                        t14[:, :, :, lo:lo + 2],
                        e4[:, :, lo:lo + 2].unsqueeze(1)
                        .to_broadcast((36, gr, 64, 2)),
                        pair[:, gs, :].unsqueeze(2)
                        .to_broadcast((36, gr, 64, 2)))
                for lo, pair in ((0, stnst), (2, spnsp)):
                    nc.vector.tensor_mul(
                        t24[:, :, :, lo:lo + 2],
                        w4[:, :, lo:lo + 2].unsqueeze(1)
                        .to_broadcast((36, gr, 64, 2)),
                        pair[:, gs, :].unsqueeze(2)
                        .to_broadcast((36, gr, 64, 2)))
                nc.vector.tensor_add(rot[:], t1[:], t2[:])
                return rot

            for b0 in range(0, BL, G):
                if b0 == 0:
                    # warm-up: two half groups so batch 0 starts sooner
                    rots = [(build_rot(0, 2, "w"), 2), (build_rot(2, 2, "w"), 2)]
                else:
                    rots = [(build_rot(b0, GR, ""), GR)]

                obuf = opool.tile([128, G * 2 * N], F16, tag="obuf", bufs=3)
                g = 0
                for rot, gr_n in rots:
                    for gr in range(gr_n):
                        b = b0 + g
                        # ---- gather: 2 fp16 matmuls, one 2-bank psum ----
                        ps = psum.tile([128, 2 * N], F32, tag="ps", bufs=4)
                        for h in range(2):
                            nc.tensor.matmul(ps[:, h * N:(h + 1) * N],
                                             rot[:, gr, h * 128:(h + 1) * 128],
                                             oh_all[:, b * N:(b + 1) * N],
                                             start=True, stop=True)
                        oslice = obuf[:, g * 2 * N:(g + 1) * 2 * N]
                        if evac_dve[b % 4]:
                            nc.vector.tensor_copy(oslice, ps[:])
                        else:
                            nc.scalar.activation(oslice, ps[:], AF.Copy)
                        if b0 + G >= BL:
                            # tail: fire each batch's write immediately
                            nc.sync.dma_start(
                                out=out[:, b * 2 * N:(b + 1) * 2 * N],
                                in_=oslice)
                        g += 1
                if b0 + G < BL:
                    nc.sync.dma_start(
                        out=out[:, b0 * 2 * N:(b0 + G) * 2 * N], in_=obuf[:])

    nc.compile()
    return nc


@functools.lru_cache(maxsize=1)
def _get_nc() -> bass.Bass:
    return build_bass()


def kernel_with_results(excitations, n_electrons, n_protons, emb_weight,
                        lookup_table, trace=False):
    exc = np.asarray(excitations)
    flat = (exc[..., 0] * 6 + exc[..., 1]).reshape(B, N)
    oh = (flat[:, None, :] == np.arange(36)[None, :, None]).astype(np.float16)
    ne = np.asarray(n_electrons, dtype=np.float32)
    npr = np.asarray(n_protons, dtype=np.float32)
    emb = np.ascontiguousarray(np.asarray(emb_weight, dtype=np.float32))
    lut_f = np.ascontiguousarray(
        np.asarray(lookup_table).astype(np.float32).reshape(1, 36))

    in_maps = []
    for c in range(N_CORES):
        sl = slice(c * BL, (c + 1) * BL)
        in_maps.append({
            "oh": np.ascontiguousarray(
                oh[sl].transpose(1, 0, 2).reshape(36, BL * N)),
            "ne": np.ascontiguousarray(ne[sl].reshape(BL, 1)),
            "npr": np.ascontiguousarray(npr[sl].reshape(BL, 1)),
            "emb": emb,
            "lut": lut_f,
        })

    nc = _get_nc()
    res = run_bass_kernel_spmd(nc, in_maps, list(range(N_CORES)), trace=trace)
    shards = []
    for c in range(N_CORES):
        arr = np.asarray(res.results[c]["out"]).reshape(128, BL, 2, N)
        shards.append(arr.transpose(1, 3, 2, 0).reshape(BL, N, D))
    out_arr = np.concatenate(shards, axis=0).astype(np.float32)
    return np.ascontiguousarray(out_arr), res


def kernel(excitations, n_electrons, n_protons, emb_weight, lookup_table):
    out_arr, _ = kernel_with_results(excitations, n_electrons, n_protons,
                                     emb_weight, lookup_table)
    return out_arr


# revision 29
# speedup vs baseline: 1.0569x; 1.0263x over previous
"""Trainium2 Bass kernel for ExcitationEmbedding + Ion RoPE.

Computes, for inputs
  excitations [256, 512, 2] int64 (pairs (a, b) with a, b in [0, 6)),
  n_electrons [256] f32, n_protons [256] f32,
  emb_weight  [26, 256] f32, lookup_table [6, 6] int64:

  idx   = lookup_table[a, b]                       # [B, N]
  emb   = emb_weight[idx]                          # [B, N, D]
  out   = per-batch block-diagonal rotation of emb (theta from n_electrons,
          phi from n_protons, 4-wide blocks: dims (0,1) by theta, (2,3) by phi)

Strategy (v4; pure data parallel over 8 cores, 32 batches each):
  - Host sends the token one-hot [36, BL*N] fp16 (pure index marshalling);
    the lut and emb tables are consumed on-device via a select-matmul that
    builds the 36-row fp16 tables e16 / esw16 (pair-swapped).
  - Per-batch rotated tables rot[j, d] = e16*C_b + esw16*S_b are built
    j-major in groups of 4 batches with 3 DVE ops per group; the C/S
    patterns reach all 36 partitions via one DRAM-bounce broadcast DMA
    covering all batches.
  - Gather: out_T[d_half, tok] = rot_slice.T @ onehot, fp16 matmuls with
    N=512 token streams, 2 per batch, weights and fmap both at partition 0.
  - PSUM pairs both halves in one [128, 1024] tile; evacuation (f32->fp16)
    alternates DVE/Act, weighted toward Act.
  - Output is fp16 in a [128, BL, 2, 512] d-major DRAM layout (8 KB
    contiguous per-partition packets, G=4 batches per sync-queue DMA); the
    host transposes back and converts to f32.
"""

import functools

import numpy as np

import concourse.bass as bass
import concourse.bacc as bacc
import concourse.mybir as mybir
from concourse import tile
from concourse.bass_utils import run_bass_kernel_spmd

B, N, D = 256, 512, 256
N_CORES = 8
BL = B // N_CORES   # 32 batches per core
G = 4               # batches per rot-group and per output DMA
ANGLE_SCALE = 0.05
HALF_PI = float(np.pi / 2)

F32 = mybir.dt.float32
F16 = mybir.dt.float16
AF = mybir.ActivationFunctionType
ALU = mybir.AluOpType


def build_bass() -> bass.Bass:
    nc = bacc.Bacc(
        "TRN2", target_bir_lowering=False, debug=False, num_devices=N_CORES
    )

    oh_in = nc.dram_tensor("oh", [36, BL * N], F16, kind="ExternalInput")
    ne = nc.dram_tensor("ne", [BL, 1], F32, kind="ExternalInput")
    npr = nc.dram_tensor("npr", [BL, 1], F32, kind="ExternalInput")
    emb = nc.dram_tensor("emb", [26, D], F32, kind="ExternalInput")
    lut = nc.dram_tensor("lut", [1, 36], F32, kind="ExternalInput")
    # out[p, b, h, n] = result[b, n, h*128 + p]
    out = nc.dram_tensor("out", [128, BL * 2 * N], F16, kind="ExternalOutput")

    iota_f32 = nc.inline_tensor(
        np.arange(36, dtype=np.float32).reshape(36, 1), "iota_f32")

    with tile.TileContext(nc) as tc:
        with (
            tc.tile_pool(name="const", bufs=1) as const,
            tc.tile_pool(name="bpool", bufs=3) as bpool,
            tc.tile_pool(name="opool", bufs=2) as opool,
            tc.tile_pool(name="dram", bufs=1, space="DRAM") as dram,
            tc.tile_pool(name="psum", bufs=4, space="PSUM") as psum,
        ):
            # ---- loads (all on sync queue so Act starts computing at
            # once; sync is otherwise idle until the first output DMA) ----
            emb_f = const.tile([26, D], F32)
            nc.sync.dma_start(out=emb_f[:], in_=emb[:])
            lut_bc = const.tile([26, 36], F32)
            nc.sync.dma_start(out=lut_bc[:],
                              in_=lut[0:1, :].to_broadcast((26, 36)))
            ne_bc = const.tile([36, BL], F32)
            nc.sync.dma_start(
                out=ne_bc[:],
                in_=ne[:].rearrange("q o -> (q o)").unsqueeze(0)
                .to_broadcast((36, BL)))
            npr_bc = const.tile([36, BL], F32)
            nc.sync.dma_start(
                out=npr_bc[:],
                in_=npr[:].rearrange("q o -> (q o)").unsqueeze(0)
                .to_broadcast((36, BL)))
            # one-hot in 4 separate tiles so batch 0 only waits on chunk 0
            QB = BL // 4
            oh_q = []
            for c in range(4):
                t = const.tile([36, QB * N], F16, tag=f"ohq{c}", bufs=1)
                nc.sync.dma_start(
                    out=t[:], in_=oh_in[:, c * QB * N:(c + 1) * QB * N])
                oh_q.append(t)
            iota_s = const.tile([36, 1], F32)
            nc.gpsimd.iota(iota_s[:], pattern=[[0, 1]], base=0,
                           channel_multiplier=1,
                           allow_small_or_imprecise_dtypes=True)

            # ---- sin/cos pair tiles [36, BL, 2] fed straight from the
            # partition-broadcast ne/npr reads (no DRAM bounce) ----
            hp36 = const.tile([36, 1], F32)
            nc.vector.memset(hp36[:], HALF_PI)
            pm2 = const.tile([36, 2], F32)
            nc.vector.memset(pm2[:, 0:1], ANGLE_SCALE)
            nc.vector.memset(pm2[:, 1:2], -ANGLE_SCALE)
            # dummy activation preloads the Sin table before ne/npr arrive
            scratch = const.tile([36, 1], F32)
            nc.scalar.activation(scratch[:], hp36[:], AF.Sin, bias=0.0,
                                 scale=1.0)
            nepm = const.tile([36, BL, 2], F32)
            nc.vector.tensor_mul(
                nepm[:], ne_bc[:].unsqueeze(2).to_broadcast((36, BL, 2)),
                pm2[:].unsqueeze(1).to_broadcast((36, BL, 2)))
            nppm = const.tile([36, BL, 2], F32)
            nc.vector.tensor_mul(
                nppm[:], npr_bc[:].unsqueeze(2).to_broadcast((36, BL, 2)),
                pm2[:].unsqueeze(1).to_broadcast((36, BL, 2)))
            ctct = const.tile([36, BL, 2], F16)
            cpcp = const.tile([36, BL, 2], F16)
            stnst = const.tile([36, BL, 2], F16)
            spnsp = const.tile([36, BL, 2], F16)
            # cos(t) = sin(pi/2 - t) keeps the LUT argument within [-pi, pi]
            nc.scalar.activation(
                ctct[:], ne_bc[:].unsqueeze(2).to_broadcast((36, BL, 2)),
                AF.Sin, bias=hp36[:], scale=-ANGLE_SCALE)
            nc.scalar.activation(
                cpcp[:], npr_bc[:].unsqueeze(2).to_broadcast((36, BL, 2)),
                AF.Sin, bias=hp36[:], scale=-ANGLE_SCALE)
            nc.scalar.activation(stnst[:], nepm[:], AF.Sin, bias=0.0,
                                 scale=1.0)
            nc.scalar.activation(spnsp[:], nppm[:], AF.Sin, bias=0.0,
                                 scale=1.0)

            # ---- 36-row fp16 tables via select-matmul ----
            emb16 = const.tile([26, D], F16)
            nc.vector.tensor_copy(emb16[:], emb_f[:])
            selT = const.tile([26, 36], F16)
            nc.vector.tensor_scalar(out=selT[:], in0=lut_bc[:],
                                    scalar1=iota_s[0:26, :], scalar2=None,
                                    op0=ALU.is_equal)
            eph_ps = psum.tile([128, 2 * N], F32, tag="ps", bufs=4)
            nc.tensor.matmul(eph_ps[0:36, 0:D], selT[:], emb16[:], start=True,
                             stop=True)
            e16 = const.tile([36, D], F16)
            nc.vector.tensor_copy(e16[:], eph_ps[0:36, 0:D])
            esw = const.tile([36, D], F16)
            e2 = e16[:].rearrange("j (k i) -> j k i", i=2)
            s2 = esw[:].rearrange("j (k i) -> j k i", i=2)
            nc.vector.tensor_copy(s2[:, :, 0], e2[:, :, 1])
            nc.vector.tensor_copy(s2[:, :, 1], e2[:, :, 0])

            # Act is faster per evac column but DVE carries the rot build:
            # 1 -> DVE, 0 -> Act (8 DVE / 24 Act out of 32)
            evac_dve = [0, 0, 1, 0]

            e4 = e16[:].rearrange("j (k i) -> j k i", i=4)
            w4 = esw[:].rearrange("j (k i) -> j k i", i=4)

            GR = 4   # batches per steady-state rot build

            def build_rot(b0, gr, tag):
                gs = slice(b0, b0 + gr)
                t1 = bpool.tile([36, gr, D], F16, tag="t1" + tag, bufs=4)
                t2 = bpool.tile([36, gr, D], F16, tag="t2" + tag, bufs=4)
                rot = bpool.tile([36, gr, D], F16, tag="rot" + tag, bufs=4)
                t14 = t1[:].rearrange("j q (k i) -> j q k i", i=4)
                t24 = t2[:].rearrange("j q (k i) -> j q k i", i=4)
                for lo, pair in ((0, ctct), (2, cpcp)):
                    nc.vector.tensor_mul(
                        t14[:, :, :, lo:lo + 2],
                        e4[:, :, lo:lo + 2].unsqueeze(1)
                        .to_broadcast((36, gr, 64, 2)),
                        pair[:, gs, :].unsqueeze(2)
                        .to_broadcast((36, gr, 64, 2)))
                for lo, pair in ((0, stnst), (2, spnsp)):
                    nc.vector.tensor_mul(
                        t24[:, :, :, lo:lo + 2],
                        w4[:, :, lo:lo + 2].unsqueeze(1)
                        .to_broadcast((36, gr, 64, 2)),
                        pair[:, gs, :].unsqueeze(2)
                        .to_broadcast((36, gr, 64, 2)))
                nc.vector.tensor_add(rot[:], t1[:], t2[:])
                return rot

            for b0 in range(0, BL, G):
                if b0 == 0:
                    # warm-up: two half groups so batch 0 starts sooner
                    rots = [(build_rot(0, 2, "w"), 2), (build_rot(2, 2, "w"), 2)]
                else:
                    rots = [(build_rot(b0, GR, ""), GR)]

                obuf = opool.tile([128, G * 2 * N], F16, tag="obuf", bufs=3)
                g = 0
                for rot, gr_n in rots:
                    for gr in range(gr_n):
                        b = b0 + g
                        # ---- gather: 2 fp16 matmuls, one 2-bank psum ----
                        ps = psum.tile([128, 2 * N], F32, tag="ps", bufs=4)
                        for h in range(2):
                            ohb = oh_q[b // QB][:, (b % QB) * N:
                                                 (b % QB + 1) * N]
                            nc.tensor.matmul(ps[:, h * N:(h + 1) * N],
                                             rot[:, gr, h * 128:(h + 1) * 128],
                                             ohb, start=True, stop=True)
                        oslice = obuf[:, g * 2 * N:(g + 1) * 2 * N]
                        if evac_dve[b % 4]:
                            nc.vector.tensor_copy(oslice, ps[:])
                        else:
                            nc.scalar.activation(oslice, ps[:], AF.Copy)
                        if b0 + G >= BL:
                            # tail: fire each batch's write immediately
                            nc.sync.dma_start(
                                out=out[:, b * 2 * N:(b + 1) * 2 * N],
                                in_=oslice)
                        g += 1
                if b0 + G < BL:
                    nc.sync.dma_start(
                        out=out[:, b0 * 2 * N:(b0 + G) * 2 * N], in_=obuf[:])

    nc.compile()
    return nc


@functools.lru_cache(maxsize=1)
def _get_nc() -> bass.Bass:
    return build_bass()


def kernel_with_results(excitations, n_electrons, n_protons, emb_weight,
                        lookup_table, trace=False):
    exc = np.asarray(excitations)
    flat = (exc[..., 0] * 6 + exc[..., 1]).reshape(B, N)
    oh = (flat[:, None, :] == np.arange(36)[None, :, None]).astype(np.float16)
    ne = np.asarray(n_electrons, dtype=np.float32)
    npr = np.asarray(n_protons, dtype=np.float32)
    emb = np.ascontiguousarray(np.asarray(emb_weight, dtype=np.float32))
    lut_f = np.ascontiguousarray(
        np.asarray(lookup_table).astype(np.float32).reshape(1, 36))

    in_maps = []
    for c in range(N_CORES):
        sl = slice(c * BL, (c + 1) * BL)
        in_maps.append({
            "oh": np.ascontiguousarray(
                oh[sl].transpose(1, 0, 2).reshape(36, BL * N)),
            "ne": np.ascontiguousarray(ne[sl].reshape(BL, 1)),
            "npr": np.ascontiguousarray(npr[sl].reshape(BL, 1)),
            "emb": emb,
            "lut": lut_f,
        })

    nc = _get_nc()
    res = run_bass_kernel_spmd(nc, in_maps, list(range(N_CORES)), trace=trace)
    shards = []
    for c in range(N_CORES):
        arr = np.asarray(res.results[c]["out"]).reshape(128, BL, 2, N)
        shards.append(arr.transpose(1, 3, 2, 0).reshape(BL, N, D))
    out_arr = np.concatenate(shards, axis=0).astype(np.float32)
    return np.ascontiguousarray(out_arr), res


def kernel(excitations, n_electrons, n_protons, emb_weight, lookup_table):
    out_arr, _ = kernel_with_results(excitations, n_electrons, n_protons,
                                     emb_weight, lookup_table)
    return out_arr


# revision 30
# speedup vs baseline: 1.0643x; 1.0070x over previous
"""Trainium2 Bass kernel for ExcitationEmbedding + Ion RoPE.

Computes, for inputs
  excitations [256, 512, 2] int64 (pairs (a, b) with a, b in [0, 6)),
  n_electrons [256] f32, n_protons [256] f32,
  emb_weight  [26, 256] f32, lookup_table [6, 6] int64:

  idx   = lookup_table[a, b]                       # [B, N]
  emb   = emb_weight[idx]                          # [B, N, D]
  out   = per-batch block-diagonal rotation of emb (theta from n_electrons,
          phi from n_protons, 4-wide blocks: dims (0,1) by theta, (2,3) by phi)

Strategy (v4; pure data parallel over 8 cores, 32 batches each):
  - Host sends the token one-hot [36, BL*N] fp16 (pure index marshalling);
    the lut and emb tables are consumed on-device via a select-matmul that
    builds the 36-row fp16 tables e16 / esw16 (pair-swapped).
  - Per-batch rotated tables rot[j, d] = e16*C_b + esw16*S_b are built
    j-major in groups of 4 batches with 3 DVE ops per group; the C/S
    patterns reach all 36 partitions via one DRAM-bounce broadcast DMA
    covering all batches.
  - Gather: out_T[d_half, tok] = rot_slice.T @ onehot, fp16 matmuls with
    N=512 token streams, 2 per batch, weights and fmap both at partition 0.
  - PSUM pairs both halves in one [128, 1024] tile; evacuation (f32->fp16)
    alternates DVE/Act, weighted toward Act.
  - Output is fp16 in a [128, BL, 2, 512] d-major DRAM layout (8 KB
    contiguous per-partition packets, G=4 batches per sync-queue DMA); the
    host transposes back and converts to f32.
"""

import functools

import numpy as np

import concourse.bass as bass
import concourse.bacc as bacc
import concourse.mybir as mybir
from concourse import tile
from concourse.bass_utils import run_bass_kernel_spmd

B, N, D = 256, 512, 256
N_CORES = 8
BL = B // N_CORES   # 32 batches per core
G = 4               # batches per rot-group and per output DMA
ANGLE_SCALE = 0.05
HALF_PI = float(np.pi / 2)

F32 = mybir.dt.float32
F16 = mybir.dt.float16
AF = mybir.ActivationFunctionType
ALU = mybir.AluOpType


def build_bass() -> bass.Bass:
    nc = bacc.Bacc(
        "TRN2", target_bir_lowering=False, debug=False, num_devices=N_CORES
    )

    oh_in = nc.dram_tensor("oh", [36, BL * N], F16, kind="ExternalInput")
    ne = nc.dram_tensor("ne", [BL, 1], F32, kind="ExternalInput")
    npr = nc.dram_tensor("npr", [BL, 1], F32, kind="ExternalInput")
    emb = nc.dram_tensor("emb", [26, D], F32, kind="ExternalInput")
    lut = nc.dram_tensor("lut", [1, 36], F32, kind="ExternalInput")
    # out[p, b, h, n] = result[b, n, h*128 + p]
    out = nc.dram_tensor("out", [128, BL * 2 * N], F16, kind="ExternalOutput")

    iota_f32 = nc.inline_tensor(
        np.arange(36, dtype=np.float32).reshape(36, 1), "iota_f32")

    with tile.TileContext(nc) as tc:
        with (
            tc.tile_pool(name="const", bufs=1) as const,
            tc.tile_pool(name="bpool", bufs=3) as bpool,
            tc.tile_pool(name="opool", bufs=2) as opool,
            tc.tile_pool(name="dram", bufs=1, space="DRAM") as dram,
            tc.tile_pool(name="psum", bufs=4, space="PSUM") as psum,
        ):
            # ---- loads (all on sync queue so Act starts computing at
            # once; sync is otherwise idle until the first output DMA) ----
            emb_f = const.tile([26, D], F32)
            nc.sync.dma_start(out=emb_f[:], in_=emb[:])
            lut_bc = const.tile([26, 36], F32)
            nc.sync.dma_start(out=lut_bc[:],
                              in_=lut[0:1, :].to_broadcast((26, 36)))
            ne_bc = const.tile([36, BL], F32)
            nc.gpsimd.dma_start(
                out=ne_bc[:],
                in_=ne[:].rearrange("q o -> (q o)").unsqueeze(0)
                .to_broadcast((36, BL)))
            npr_bc = const.tile([36, BL], F32)
            nc.gpsimd.dma_start(
                out=npr_bc[:],
                in_=npr[:].rearrange("q o -> (q o)").unsqueeze(0)
                .to_broadcast((36, BL)))
            # one-hot in 4 separate tiles so batch 0 only waits on chunk 0
            QB = BL // 4
            oh_q = []
            for c in range(4):
                t = const.tile([36, QB * N], F16, tag=f"ohq{c}", bufs=1)
                nc.sync.dma_start(
                    out=t[:], in_=oh_in[:, c * QB * N:(c + 1) * QB * N])
                oh_q.append(t)
            iota_s = const.tile([36, 1], F32)
            nc.gpsimd.iota(iota_s[:], pattern=[[0, 1]], base=0,
                           channel_multiplier=1,
                           allow_small_or_imprecise_dtypes=True)

            # ---- sin/cos pair tiles [36, BL, 2] fed straight from the
            # partition-broadcast ne/npr reads (no DRAM bounce) ----
            hp36 = const.tile([36, 1], F32)
            nc.vector.memset(hp36[:], HALF_PI)
            pm2 = const.tile([36, 2], F32)
            nc.vector.memset(pm2[:, 0:1], ANGLE_SCALE)
            nc.vector.memset(pm2[:, 1:2], -ANGLE_SCALE)
            # dummy activation preloads the Sin table before ne/npr arrive
            scratch = const.tile([36, 1], F32)
            nc.scalar.activation(scratch[:], hp36[:], AF.Sin, bias=0.0,
                                 scale=1.0)
            nepm = const.tile([36, BL, 2], F32)
            nc.vector.tensor_mul(
                nepm[:], ne_bc[:].unsqueeze(2).to_broadcast((36, BL, 2)),
                pm2[:].unsqueeze(1).to_broadcast((36, BL, 2)))
            nppm = const.tile([36, BL, 2], F32)
            nc.vector.tensor_mul(
                nppm[:], npr_bc[:].unsqueeze(2).to_broadcast((36, BL, 2)),
                pm2[:].unsqueeze(1).to_broadcast((36, BL, 2)))
            ctct = const.tile([36, BL, 2], F16)
            cpcp = const.tile([36, BL, 2], F16)
            stnst = const.tile([36, BL, 2], F16)
            spnsp = const.tile([36, BL, 2], F16)
            # cos(t) = sin(pi/2 - t) keeps the LUT argument within [-pi, pi]
            nc.scalar.activation(
                ctct[:], ne_bc[:].unsqueeze(2).to_broadcast((36, BL, 2)),
                AF.Sin, bias=hp36[:], scale=-ANGLE_SCALE)
            nc.scalar.activation(
                cpcp[:], npr_bc[:].unsqueeze(2).to_broadcast((36, BL, 2)),
                AF.Sin, bias=hp36[:], scale=-ANGLE_SCALE)
            nc.scalar.activation(stnst[:], nepm[:], AF.Sin, bias=0.0,
                                 scale=1.0)
            nc.scalar.activation(spnsp[:], nppm[:], AF.Sin, bias=0.0,
                                 scale=1.0)

            # ---- 36-row fp16 tables via select-matmul ----
            emb16 = const.tile([26, D], F16)
            nc.vector.tensor_copy(emb16[:], emb_f[:])
            selT = const.tile([26, 36], F16)
            nc.vector.tensor_scalar(out=selT[:], in0=lut_bc[:],
                                    scalar1=iota_s[0:26, :], scalar2=None,
                                    op0=ALU.is_equal)
            eph_ps = psum.tile([128, 2 * N], F32, tag="ps", bufs=4)
            nc.tensor.matmul(eph_ps[0:36, 0:D], selT[:], emb16[:], start=True,
                             stop=True)
            e16 = const.tile([36, D], F16)
            nc.vector.tensor_copy(e16[:], eph_ps[0:36, 0:D])
            esw = const.tile([36, D], F16)
            e2 = e16[:].rearrange("j (k i) -> j k i", i=2)
            s2 = esw[:].rearrange("j (k i) -> j k i", i=2)
            nc.vector.tensor_copy(s2[:, :, 0], e2[:, :, 1])
            nc.vector.tensor_copy(s2[:, :, 1], e2[:, :, 0])

            # Act is faster per evac column but DVE carries the rot build:
            # 1 -> DVE, 0 -> Act (8 DVE / 24 Act out of 32)
            evac_dve = [0, 0, 1, 0]

            e4 = e16[:].rearrange("j (k i) -> j k i", i=4)
            w4 = esw[:].rearrange("j (k i) -> j k i", i=4)

            GR = 4   # batches per steady-state rot build

            def build_rot(b0, gr, tag):
                gs = slice(b0, b0 + gr)
                t1 = bpool.tile([36, gr, D], F16, tag="t1" + tag, bufs=4)
                t2 = bpool.tile([36, gr, D], F16, tag="t2" + tag, bufs=4)
                rot = bpool.tile([36, gr, D], F16, tag="rot" + tag, bufs=4)
                t14 = t1[:].rearrange("j q (k i) -> j q k i", i=4)
                t24 = t2[:].rearrange("j q (k i) -> j q k i", i=4)
                for lo, pair in ((0, ctct), (2, cpcp)):
                    nc.vector.tensor_mul(
                        t14[:, :, :, lo:lo + 2],
                        e4[:, :, lo:lo + 2].unsqueeze(1)
                        .to_broadcast((36, gr, 64, 2)),
                        pair[:, gs, :].unsqueeze(2)
                        .to_broadcast((36, gr, 64, 2)))
                for lo, pair in ((0, stnst), (2, spnsp)):
                    nc.vector.tensor_mul(
                        t24[:, :, :, lo:lo + 2],
                        w4[:, :, lo:lo + 2].unsqueeze(1)
                        .to_broadcast((36, gr, 64, 2)),
                        pair[:, gs, :].unsqueeze(2)
                        .to_broadcast((36, gr, 64, 2)))
                nc.vector.tensor_add(rot[:], t1[:], t2[:])
                return rot

            for b0 in range(0, BL, G):
                if b0 == 0:
                    # warm-up: batch 0 alone so its matmul starts sooner
                    rots = [(build_rot(0, 1, "w1"), 1),
                            (build_rot(1, 3, "w3"), 3)]
                else:
                    rots = [(build_rot(b0, GR, ""), GR)]

                obuf = opool.tile([128, G * 2 * N], F16, tag="obuf", bufs=3)
                g = 0
                for rot, gr_n in rots:
                    for gr in range(gr_n):
                        b = b0 + g
                        # ---- gather: 2 fp16 matmuls, one 2-bank psum ----
                        ps = psum.tile([128, 2 * N], F32, tag="ps", bufs=4)
                        for h in range(2):
                            ohb = oh_q[b // QB][:, (b % QB) * N:
                                                 (b % QB + 1) * N]
                            nc.tensor.matmul(ps[:, h * N:(h + 1) * N],
                                             rot[:, gr, h * 128:(h + 1) * 128],
                                             ohb, start=True, stop=True)
                        oslice = obuf[:, g * 2 * N:(g + 1) * 2 * N]
                        if evac_dve[b % 4]:
                            nc.vector.tensor_copy(oslice, ps[:])
                        else:
                            nc.scalar.activation(oslice, ps[:], AF.Copy)
                        if b0 + G >= BL:
                            # tail: fire each batch's write immediately
                            nc.sync.dma_start(
                                out=out[:, b * 2 * N:(b + 1) * 2 * N],
                                in_=oslice)
                        g += 1
                if b0 + G < BL:
                    nc.sync.dma_start(
                        out=out[:, b0 * 2 * N:(b0 + G) * 2 * N], in_=obuf[:])

    nc.compile()
    return nc


@functools.lru_cache(maxsize=1)
def _get_nc() -> bass.Bass:
    return build_bass()


def kernel_with_results(excitations, n_electrons, n_protons, emb_weight,
                        lookup_table, trace=False):
    exc = np.asarray(excitations)
    flat = (exc[..., 0] * 6 + exc[..., 1]).reshape(B, N)
    oh = (flat[:, None, :] == np.arange(36)[None, :, None]).astype(np.float16)
    ne = np.asarray(n_electrons, dtype=np.float32)
    npr = np.asarray(n_protons, dtype=np.float32)
    emb = np.ascontiguousarray(np.asarray(emb_weight, dtype=np.float32))
    lut_f = np.ascontiguousarray(
        np.asarray(lookup_table).astype(np.float32).reshape(1, 36))

    in_maps = []
    for c in range(N_CORES):
        sl = slice(c * BL, (c + 1) * BL)
        in_maps.append({
            "oh": np.ascontiguousarray(
                oh[sl].transpose(1, 0, 2).reshape(36, BL * N)),
            "ne": np.ascontiguousarray(ne[sl].reshape(BL, 1)),
            "npr": np.ascontiguousarray(npr[sl].reshape(BL, 1)),
            "emb": emb,
            "lut": lut_f,
        })

    nc = _get_nc()
    res = run_bass_kernel_spmd(nc, in_maps, list(range(N_CORES)), trace=trace)
    shards = []
    for c in range(N_CORES):
        arr = np.asarray(res.results[c]["out"]).reshape(128, BL, 2, N)
        shards.append(arr.transpose(1, 3, 2, 0).reshape(BL, N, D))
    out_arr = np.concatenate(shards, axis=0).astype(np.float32)
    return np.ascontiguousarray(out_arr), res


def kernel(excitations, n_electrons, n_protons, emb_weight, lookup_table):
    out_arr, _ = kernel_with_results(excitations, n_electrons, n_protons,
                                     emb_weight, lookup_table)
    return out_arr
